# revision 44
# baseline (speedup 1.0000x reference)
"""Trainium2 Bass kernel for the self-attention block (nn_Attention).

Reference computation (per batch b, row h):
    f = x @ wf + bf; g = x @ wg + bg; h = x @ wh + bh      (1x1 convs)
    s = g @ f^T (over W); beta = softmax(s, -1); o = beta @ h
    out = gamma * o + x

Sharding: data-parallel over batch B=8, one batch element per NeuronCore.
Per core, each of the 128 rows is an independent [W=128, C=512] block.

v6 (on top of v4's fp8 DoubleRow matmuls + pipelined pair loop):
  - f and g projections MERGED into one M=128 stationary [wf|wg]: 2 DR
    matmuls per rb instead of 4 (the old M=64 ones computed zeros on
    half the PE array), one full-height [128,512] drain per rb with a
    stacked [bf;bg] bias instead of two half-height ones.
  - The s-matmul needs f and g at the same base partition, but the
    merged drain leaves g in partitions 64..127.  A small SBUF->SBUF
    DMA (64KB/rb on the HW ring) relocates the g half to partitions
    0..63 -- DMA is the only engine that can remap partitions.
  - All input/output DMAs ride the sync HW-DGE ring (the gpsimd
    software-DGE path measured ~2.5us per 256KB and starved the PE);
    ring order puts xt8(rb0)+wf+wg first; inputs prefetch ~2 rbs ahead.
  - h PSUM drained entirely by ACT; fg drain on DVE (tensor_scalar
    bias-add); DVE also keeps the stt epilogue + reciprocal.
  - ~6us of dummy matmuls bridge the runtime's ~9us DMA-startup delay
    and trip the PE HAM activity window (cold default is 1.2GHz).
"""

import numpy as np
import ml_dtypes

import concourse.bacc as bacc
import concourse.bass as bass
import concourse.mybir as mybir
import concourse.tile as tile

B, H, W, C = 8, 128, 128, 512
CK = C // 8  # 64
N_CORES = 8

F32 = mybir.dt.float32
BF16 = mybir.dt.bfloat16
FP8 = mybir.dt.float8e4
BFDT = ml_dtypes.bfloat16
E4DT = ml_dtypes.float8_e4m3
AF = mybir.ActivationFunctionType
ALU = mybir.AluOpType
DR = mybir.MatmulPerfMode.DoubleRow

WS = 64.0    # host-side weight scale


def row_batch(rows: int) -> int:
    for rb in (4, 2):
        if rows % rb == 0:
            return rb
    return 1


def build_nc(rows: int = H) -> bass.Bass:
    nc = bacc.Bacc(None)
    RB = row_batch(rows)
    nrb = rows // RB
    npair = RB // 2
    assert npair, "rows must be a multiple of 2"

    xt8_d = nc.dram_tensor("xt8", [nrb, 128, RB * C], FP8, kind="ExternalInput")
    x4_d = nc.dram_tensor("x4", [nrb, 128, RB * C], BF16, kind="ExternalInput")
    wh8_d = nc.dram_tensor("wh8", [128, 2 * 2 * C], FP8, kind="ExternalInput")
    wfg8_d = nc.dram_tensor("wfg8", [128, 2 * 2 * 128], FP8, kind="ExternalInput")
    bfg_d = nc.dram_tensor("bfg", [128, 1], F32, kind="ExternalInput")
    onesg_d = nc.dram_tensor("onesg", [W, 1], BF16, kind="ExternalInput")
    out_d = nc.dram_tensor("out", [nrb, 128, RB * C], BF16, kind="ExternalOutput")

    with tile.TileContext(nc) as tc:
        with (
            tc.tile_pool(name="const", bufs=1) as cpool,
            tc.tile_pool(name="sb_xt", bufs=6) as sb_xt,
            tc.tile_pool(name="sb_x", bufs=6) as sb_x,
            tc.tile_pool(name="sb_fg", bufs=3) as sb_fg,
            tc.tile_pool(name="sb_gt", bufs=3) as sb_gt,
            tc.tile_pool(name="sb_h", bufs=5) as sb_h,
            tc.tile_pool(name="sb_at", bufs=4) as sb_at,
            tc.tile_pool(name="sb_out", bufs=4) as sb_out,
            tc.tile_pool(name="sb_small", bufs=6) as sb_small,
            tc.tile_pool(name="ps_fg", bufs=1, space="PSUM") as ps_fg,
            tc.tile_pool(name="ps_h", bufs=3, space="PSUM") as ps_h,
            tc.tile_pool(name="ps_s", bufs=2, space="PSUM") as ps_s,
            tc.tile_pool(name="ps_o", bufs=2, space="PSUM") as ps_o,
        ):
            wh8_sb = cpool.tile([128, 2, 2, C], FP8)
            wfg8_sb = cpool.tile([128, 2, 2, 128], FP8)
            bfg_sb = cpool.tile([128, 1], F32)
            onesg_sb = cpool.tile([W, 1], BF16)

            def start_dma(rb):
                """Input DMAs for one rb (4 rows).  xt8 rides the gpsimd
                software-DGE queue (its ~2.5us latency is hidden by the
                2.5-rb prefetch lead and it keeps the sync HW ring, which
                measured 86% busy, off the critical path); x4 on sync."""
                st = {"rb": rb}
                st["xt8"] = sb_xt.tile(
                    [128, 2, 2, RB, 128], FP8, tag="xt8", name="xt8_t"
                )
                nc.sync.dma_start(st["xt8"][:], xt8_d[rb])
                st["x4"] = sb_x.tile([128, RB * C], BF16, tag="x4", name="x4_t")
                nc.sync.dma_start(st["x4"][:], x4_d[rb])
                st["out4"] = sb_out.tile(
                    [128, RB * C], BF16, tag="out4", name="out4_t"
                )
                return st

            def fg_stage(st):
                """Merged f/g projection for one rb: 2 DR matmuls + one
                full-height DVE drain (bias via tensor_scalar)."""
                fgA = ps_fg.tile([128, RB * 128], F32, tag="fgA", name="fgA_t")
                for j in range(2):
                    nc.tensor.matmul(
                        fgA[:, :], lhsT=wfg8_sb[:, j], rhs=st["xt8"][:, j],
                        start=(j == 0), stop=(j == 1), perf_mode=DR,
                    )
                fg = sb_fg.tile([128, RB, 128], BF16, tag="fg", name="fg_t")
                nc.vector.tensor_scalar(fg[:], fgA[:], bfg_sb[:], None, ALU.add)
                st["ft"] = fg

            def g_remap(st):
                """DMA partition-remap of the g half to partitions 0..63
                (issued a pair after the drain so the ring never blocks)."""
                gt = sb_gt.tile([64, RB, 128], BF16, tag="gt", name="gt_t")
                nc.sync.dma_start(gt[:], st["ft"][64:128])
                st["gt"] = gt

            def emit_h(st, p):
                """h matmuls + full-ACT drains for pair p of rb st."""
                h2s = []
                for rr in range(2):
                    hp = ps_h.tile([128, C], F32, tag="h", name="h_ps")
                    for j in range(2):
                        nc.tensor.matmul(
                            hp[:], lhsT=st["xt8"][:, j, :, 2 * p + rr, :],
                            rhs=wh8_sb[:, j],
                            start=(j == 0), stop=(j == 1), perf_mode=DR,
                        )
                    h2 = sb_h.tile([128, C], BF16, tag="h2", name="h2_t")
                    nc.scalar.activation(h2[:], hp[:], AF.Identity)
                    h2s.append(h2)
                return h2s

            def stage_b1(e):
                """Z-matmuls + reciprocal for a pair whose exp already ran."""
                s_ps, at2 = e["s_ps"], e["at2"]
                for rr in range(2):
                    nc.tensor.matmul(
                        s_ps[:, 256 + rr : 257 + rr],
                        lhsT=at2[:, rr * 128 : (rr + 1) * 128],
                        rhs=onesg_sb[:],
                        start=True, stop=True,
                    )
                scale2 = sb_small.tile([128, 2], F32, tag="scale2", name="sc_t")
                nc.vector.reciprocal(scale2[:], s_ps[:, 256:258])
                e["scale2"] = scale2

            def stage_b2(e):
                """o matmuls + epilogue."""
                st, p, at2, h2, scale2 = e["st"], e["p"], e["at2"], e["h2"], e["scale2"]
                for rr in range(2):
                    r = 2 * p + rr
                    o_ps = ps_o.tile([128, C], F32, tag="o", name="o_ps")
                    nc.tensor.matmul(
                        o_ps[:], lhsT=at2[:, rr * 128 : (rr + 1) * 128],
                        rhs=h2[rr][:], start=True, stop=True,
                    )
                    nc.vector.scalar_tensor_tensor(
                        st["out4"][:, r * C : (r + 1) * C],
                        o_ps[:],
                        scale2[:, rr : rr + 1],
                        st["x4"][:, r * C : (r + 1) * C],
                        ALU.mult,
                        ALU.add,
                    )
                if p == npair - 1:
                    pending_out.append(st)
                    # flush the PREVIOUS rb's output: its stt is long done,
                    # so the ring trigger never blocks waiting for data
                    if len(pending_out) > 1:
                        po = pending_out.pop(0)
                        nc.sync.dma_start(out_d[po["rb"]], po["out4"][:])

            # ---- prologue ----
            # ring order: xt8(0) + f/g weights first (first matmuls), then
            # wh8 (first emit_h), bias/ones, x4(0), then rb1/rb2 inputs.
            sts = {}
            st0 = {"rb": 0}
            st0["xt8"] = sb_xt.tile([128, 2, 2, RB, 128], FP8, tag="xt8",
                                    name="xt8_t")
            nc.sync.dma_start(st0["xt8"][:], xt8_d[0])
            nc.sync.dma_start(wfg8_sb[:], wfg8_d[:])
            nc.sync.dma_start(bfg_sb[:], bfg_d[:])
            nc.sync.dma_start(wh8_sb[:], wh8_d[:])
            nc.sync.dma_start(onesg_sb[:], onesg_d[:])
            st0["x4"] = sb_x.tile([128, RB * C], BF16, tag="x4", name="x4_t")
            st0["out4"] = sb_out.tile([128, RB * C], BF16, tag="out4",
                                      name="out4_t")
            nc.sync.dma_start(st0["x4"][:], x4_d[0])
            sts[0] = st0
            for rb in (1, 2):
                if rb < nrb:
                    sts[rb] = start_dma(rb)
            # Dummy matmuls bridging the runtime's ~9us DMA-startup delay:
            # trip the PE HAM activity window so real work starts at 2.4GHz.
            wu = cpool.tile([128, 128], BF16)
            nc.vector.memset(wu[:], 0.0)
            wu_ps = ps_s.tile([128, 258], F32, tag="s", name="wu_ps")
            for _ in range(56):
                nc.tensor.matmul(
                    wu_ps[0:8, 0:128], lhsT=wu[:, 0:8], rhs=wu[:, :],
                    start=True, stop=True,
                )
            fg_stage(sts[0])
            if nrb > 1:
                fg_stage(sts[1])
            g_remap(sts[0])
            pending_out = []

            pairs = [(rb, p) for rb in range(nrb) for p in range(npair)]
            cur_st = sts[0]
            h2_next = emit_h(cur_st, 0)
            prev = None
            for i, (rb, p) in enumerate(pairs):
                st = sts[rb]
                ft, gt = st["ft"], st["gt"]
                h2_this = h2_next
                s_ps = ps_s.tile([128, 258], F32, tag="s", name="s_ps")
                for rr in range(2):
                    r = 2 * p + rr
                    nc.tensor.matmul(
                        s_ps[:, rr * 128 : (rr + 1) * 128],
                        lhsT=ft[0:64, r], rhs=gt[:, r],
                        start=True, stop=True,
                    )
                # next rb's merged f/g matmuls right behind the s-matmuls;
                # drain at p0, g-remap a pair later (dep already satisfied)
                if p == 0 and rb + 2 < nrb:
                    fg_stage(sts[rb + 2])
                # remap for rb+1: its drain ran 3+ pairs ago, so the ring
                # trigger's dependency wait is already satisfied
                if p == npair - 1 and rb + 1 < nrb and "gt" not in sts[rb + 1]:
                    g_remap(sts[rb + 1])
                at2 = sb_at.tile([128, 256], BF16, tag="at2", name="at2_t")
                nc.scalar.activation(
                    at2[:], s_ps[:, 0:256], AF.Exp, scale=1.0 / (WS * WS)
                )
                # input DMAs ~2.5 rbs ahead
                if p == npair - 1 and rb + 3 < nrb:
                    sts[rb + 3] = start_dma(rb + 3)

                if i + 1 < len(pairs):
                    nrb2, np2 = pairs[i + 1]
                    h2_next = emit_h(sts[nrb2], np2)
                else:
                    h2_next = None

                if prev is not None:
                    stage_b1(prev)
                    stage_b2(prev)
                prev = {"st": st, "p": p, "s_ps": s_ps, "at2": at2, "h2": h2_this}
            stage_b1(prev)
            stage_b2(prev)
            for po in pending_out:
                nc.sync.dma_start(out_d[po["rb"]], po["out4"][:])
    nc.compile()
    return nc


def make_in_map(x_b: np.ndarray, wf, bf, wg, bg, wh, bh, gamma) -> dict:
    """Host-side input staging for one core (layout/dtype + constant folds)."""
    x_b = np.asarray(x_b, np.float32)
    rows = x_b.shape[0]
    RB = row_batch(rows)
    nrb = rows // RB
    gamma_f = float(np.float32(np.asarray(gamma)))
    sgn = 1.0 if gamma_f >= 0 else -1.0
    ag = max(abs(gamma_f), 1e-30)

    xt8 = np.ascontiguousarray(
        x_b.astype(E4DT)
        .reshape(nrb, RB, W, 4, 128)
        .transpose(0, 4, 3, 1, 2)
        .reshape(nrb, 128, RB * C)
    )
    x_adj = x_b + gamma_f * np.asarray(bh, np.float32)
    x4 = np.ascontiguousarray(
        x_adj.astype(BFDT)
        .reshape(nrb, RB, W, C)
        .transpose(0, 2, 1, 3)
        .reshape(nrb, 128, RB * C)
    )

    def w_dr(w_mat, scale):
        w_mat = np.asarray(w_mat, np.float32) * scale
        m = w_mat.shape[1]
        return np.ascontiguousarray(
            w_mat.astype(E4DT).reshape(4, 128, m).transpose(1, 0, 2).reshape(128, 4 * m)
        )

    wfg = np.concatenate(
        [np.asarray(wf, np.float32), np.asarray(wg, np.float32)], axis=1
    )
    bfg = np.concatenate(
        [np.asarray(bf, np.float32), np.asarray(bg, np.float32)]
    ).reshape(128, 1) * WS
    return {
        "xt8": xt8,
        "x4": x4,
        "wh8": w_dr(wh, WS * sgn),
        "wfg8": w_dr(wfg, WS),
        "bfg": bfg,
        "onesg": np.full((W, 1), WS / ag, np.float32).astype(BFDT),
    }


def unbatch_out(arr: np.ndarray, rows: int) -> np.ndarray:
    """[nrb, 128, RB*C] device layout -> [rows, W, C] f32."""
    RB = row_batch(rows)
    nrb = rows // RB
    return (
        np.asarray(arr)
        .astype(np.float32)
        .reshape(nrb, 128, RB, C)
        .transpose(0, 2, 1, 3)
        .reshape(rows, W, C)
    )


_NC_CACHE: dict = {}


def run(inputs: dict, trace: bool = False, **run_kwargs):
    """Build (cached), run on 8 cores, return (out, BassKernelResults)."""
    from concourse.bass_utils import run_bass_kernel_spmd

    if "nc" not in _NC_CACHE:
        _NC_CACHE["nc"] = build_nc()
    nc = _NC_CACHE["nc"]
    x = np.asarray(inputs["x"], np.float32)
    in_maps = [
        make_in_map(
            x[b],
            inputs["wf"],
            inputs["bf"],
            inputs["wg"],
            inputs["bg"],
            inputs["wh"],
            inputs["bh"],
            inputs["gamma"],
        )
        for b in range(N_CORES)
    ]
    res = run_bass_kernel_spmd(
        nc, in_maps, list(range(N_CORES)), trace=trace, **run_kwargs
    )
    out = np.stack(
        [unbatch_out(res.results[b]["out"], H) for b in range(N_CORES)], axis=0
    )
    return out, res


def kernel(**inputs) -> np.ndarray:
    out, _ = run(inputs, trace=False)
    return out


# revision 45
# speedup vs baseline: 1.1503x; 1.1503x over previous
"""Trainium2 Bass kernel for the self-attention block (nn_Attention).

Reference computation (per batch b, row h):
    f = x @ wf + bf; g = x @ wg + bg; h = x @ wh + bh      (1x1 convs)
    s = g @ f^T (over W); beta = softmax(s, -1); o = beta @ h
    out = gamma * o + x

Sharding: data-parallel over batch B=8, one batch element per NeuronCore.
Per core, each of the 128 rows is an independent [W=128, C=512] block.

v5 (on top of v4's fp8 DoubleRow matmuls + pipelined pair loop):
  - f/g projections of an EVEN/ODD rb pair write the two partition halves
    of shared PSUM banks (odd rb via output col-tiling to partitions
    64..127), so the four half-height [64,512] drains per 2 rbs become
    one full-height [128,512] ACT op (f, with stacked bias) plus one DVE
    tensor_scalar bias-add (g).  s-matmuls slice the parity half; both
    operands share a base partition, which HW requires.
  - h PSUM is drained entirely by ACT (512 cols); DVE keeps the stt
    epilogue + reciprocal + g drain, which measured/estimated balances
    the two engines (~2.0us/pair each).
  - x4's dma_start moved from the Scalar queue to Sync (ACT was the
    co-bottleneck; a 600ns DMA trigger per rb was ~300ns/pair of ACT).
"""

import numpy as np
import ml_dtypes

import concourse.bacc as bacc
import concourse.bass as bass
import concourse.mybir as mybir
import concourse.tile as tile

B, H, W, C = 8, 128, 128, 512
CK = C // 8  # 64
N_CORES = 8

F32 = mybir.dt.float32
BF16 = mybir.dt.bfloat16
FP8 = mybir.dt.float8e4
BFDT = ml_dtypes.bfloat16
E4DT = ml_dtypes.float8_e4m3
AF = mybir.ActivationFunctionType
ALU = mybir.AluOpType
DR = mybir.MatmulPerfMode.DoubleRow

WS = 64.0    # host-side weight scale


def row_batch(rows: int) -> int:
    for rb in (4, 2):
        if rows % rb == 0:
            return rb
    return 1


def build_nc(rows: int = H) -> bass.Bass:
    nc = bacc.Bacc(None)
    RB = row_batch(rows)
    nrb = rows // RB
    npair = RB // 2
    assert npair, "rows must be a multiple of 2"
    # supers: groups of (up to) 2 rbs sharing f/g PSUM banks
    supers = [list(range(s, min(s + 2, nrb))) for s in range(0, nrb, 2)]

    xt8_d = nc.dram_tensor("xt8", [nrb, 128, RB * C], FP8, kind="ExternalInput")
    x4_d = nc.dram_tensor("x4", [nrb, 128, RB * C], BF16, kind="ExternalInput")
    wh8_d = nc.dram_tensor("wh8", [128, 2 * 2 * C], FP8, kind="ExternalInput")
    # f/g weights zero-padded along M to 192 cols: [0(64) | w(64) | 0(64)].
    # The even rb of a super uses the [64:192] slice = [w|0] (start=True:
    # writes w into psum parts 0..63 and zeros into 64..127); the odd rb
    # uses [0:128] = [0|w] with start=False (accumulates +0 / +w).  This
    # shares one psum bank between the two rbs so ONE full-height 128-
    # partition drain replaces two half-height ones.  (DR matmuls may only
    # target psum partition 0, so output col-tiling was not an option.)
    wf8_d = nc.dram_tensor("wf8", [128, 2 * 2 * 192], FP8, kind="ExternalInput")
    wg8_d = nc.dram_tensor("wg8", [128, 2 * 2 * 192], FP8, kind="ExternalInput")
    bf2_d = nc.dram_tensor("bf2", [128, 1], F32, kind="ExternalInput")
    bg2_d = nc.dram_tensor("bg2", [128, 1], F32, kind="ExternalInput")
    onesg_d = nc.dram_tensor("onesg", [W, 1], BF16, kind="ExternalInput")
    out_d = nc.dram_tensor("out", [nrb, 128, RB * C], BF16, kind="ExternalOutput")

    with tile.TileContext(nc) as tc:
        with (
            tc.tile_pool(name="const", bufs=1) as cpool,
            tc.tile_pool(name="sb_xt", bufs=7) as sb_xt,
            tc.tile_pool(name="sb_x", bufs=7) as sb_x,
            tc.tile_pool(name="sb_fg", bufs=2) as sb_fg,
            tc.tile_pool(name="sb_h", bufs=5) as sb_h,
            tc.tile_pool(name="sb_at", bufs=4) as sb_at,
            tc.tile_pool(name="sb_out", bufs=3) as sb_out,
            tc.tile_pool(name="sb_small", bufs=6) as sb_small,
            tc.tile_pool(name="ps_fg", bufs=1, space="PSUM") as ps_fg,
            tc.tile_pool(name="ps_h", bufs=2, space="PSUM") as ps_h,
            tc.tile_pool(name="ps_s", bufs=2, space="PSUM") as ps_s,
            tc.tile_pool(name="ps_o", bufs=2, space="PSUM") as ps_o,
        ):
            # ring order tuned for the critical path: the first f/g matmuls
            # need xt8(rb0)+wf+wg, so those transfer first; wh8 (256KB)
            # before the first emit_h; x4/onesg only matter pairs later.
            wh8_sb = cpool.tile([128, 2, 2, C], FP8)
            wf8_sb = cpool.tile([128, 2, 2, 192], FP8)
            wg8_sb = cpool.tile([128, 2, 2, 192], FP8)
            bf2_sb = cpool.tile([128, 1], F32)
            bg2_sb = cpool.tile([128, 1], F32)
            onesg_sb = cpool.tile([W, 1], BF16)

            def start_dma(rb):
                """Input DMAs for one rb (4 rows).  All on the sync HW-DGE
                ring: the software (gpsimd) DGE path measured ~2.5us per
                256KB transfer and starved the f/g matmuls."""
                st = {"rb": rb, "par": rb & 1}
                st["xt8"] = sb_xt.tile(
                    [128, 2, 2, RB, 128], FP8, tag="xt8", name="xt8_t"
                )
                nc.sync.dma_start(st["xt8"][:], xt8_d[rb])
                st["x4"] = sb_x.tile([128, RB * C], BF16, tag="x4", name="x4_t")
                nc.sync.dma_start(st["x4"][:], x4_d[rb])
                st["out4"] = sb_out.tile(
                    [128, RB * C], BF16, tag="out4", name="out4_t"
                )
                return st

            def new_fg_banks():
                fA2 = ps_fg.tile([128, RB * 128], F32, tag="fA", name="fA_t")
                gA2 = ps_fg.tile([128, RB * 128], F32, tag="gA", name="gA_t")
                return fA2, gA2

            def fg_mms(banks, st, last):
                """f/g projection matmuls for one rb into the super's shared
                psum banks via zero-padded M=128 weights (see wf8_d)."""
                fA2, gA2 = banks
                par = st["par"]
                msl = slice(64, 192) if par == 0 else slice(0, 128)
                for w_sb, bank in ((wf8_sb, fA2), (wg8_sb, gA2)):
                    for j in range(2):
                        nc.tensor.matmul(
                            bank[:, :], lhsT=w_sb[:, j, :, msl],
                            rhs=st["xt8"][:, j],
                            start=(par == 0 and j == 0),
                            stop=(j == 1) and last,
                            perf_mode=DR,
                        )

            def fg_drain(banks, nrb_in_super):
                """One full-height drain per bank: f on ACT (bias via
                activation), g on DVE (bias via tensor_scalar add)."""
                fA2, gA2 = banks
                hi = 64 * nrb_in_super
                ft = sb_fg.tile([128, RB, 128], BF16, tag="ft", name="ft_t")
                gt = sb_fg.tile([128, RB, 128], BF16, tag="gt", name="gt_t")
                nc.scalar.activation(
                    ft[0:hi], fA2[0:hi, :], AF.Identity, bias=bf2_sb[0:hi]
                )
                nc.vector.tensor_scalar(
                    gt[0:hi], gA2[0:hi, :], bg2_sb[0:hi], None, ALU.add
                )
                return ft, gt

            def emit_h(st, p):
                """h matmuls + full-ACT per-row drains for pair p of rb st.
                (A merged [128,2C] drain on a bufs=1 2-bank tile measured
                WORSE: it interlocks consecutive pairs' h matmuls/drains.)"""
                h2s = []
                for rr in range(2):
                    hp = ps_h.tile([128, C], F32, tag="h", name="h_ps")
                    for j in range(2):
                        nc.tensor.matmul(
                            hp[:], lhsT=st["xt8"][:, j, :, 2 * p + rr, :],
                            rhs=wh8_sb[:, j],
                            start=(j == 0), stop=(j == 1), perf_mode=DR,
                        )
                    h2 = sb_h.tile([128, C], BF16, tag="h2", name="h2_t")
                    nc.scalar.activation(h2[:], hp[:], AF.Identity)
                    h2s.append(h2)
                return h2s

            def stage_b1(e):
                """Z-matmuls + reciprocal for a pair whose exp already ran."""
                s_ps, at2 = e["s_ps"], e["at2"]
                for rr in range(2):
                    nc.tensor.matmul(
                        s_ps[:, 256 + rr : 257 + rr],
                        lhsT=at2[:, rr * 128 : (rr + 1) * 128],
                        rhs=onesg_sb[:],
                        start=True, stop=True,
                    )
                scale2 = sb_small.tile([128, 2], F32, tag="scale2", name="sc_t")
                nc.vector.reciprocal(scale2[:], s_ps[:, 256:258])
                e["scale2"] = scale2

            def stage_b2(e):
                """o matmuls + epilogue."""
                st, p, at2, h2, scale2 = e["st"], e["p"], e["at2"], e["h2"], e["scale2"]
                for rr in range(2):
                    r = 2 * p + rr
                    o_ps = ps_o.tile([128, C], F32, tag="o", name="o_ps")
                    nc.tensor.matmul(
                        o_ps[:], lhsT=at2[:, rr * 128 : (rr + 1) * 128],
                        rhs=h2[rr][:], start=True, stop=True,
                    )
                    nc.vector.scalar_tensor_tensor(
                        st["out4"][:, r * C : (r + 1) * C],
                        o_ps[:],
                        scale2[:, rr : rr + 1],
                        st["x4"][:, r * C : (r + 1) * C],
                        ALU.mult,
                        ALU.add,
                    )
                if p == npair - 1:
                    nc.sync.dma_start(out_d[st["rb"]], st["out4"][:])

            # ---- prologue: super 0 fully staged, super 1 DMAs in flight ----
            sts = {}
            st0 = {"rb": supers[0][0], "par": 0}
            st0["xt8"] = sb_xt.tile([128, 2, 2, RB, 128], FP8, tag="xt8",
                                    name="xt8_t")
            nc.sync.dma_start(st0["xt8"][:], xt8_d[supers[0][0]])
            nc.sync.dma_start(wf8_sb[:], wf8_d[:])
            nc.sync.dma_start(wg8_sb[:], wg8_d[:])
            nc.sync.dma_start(bf2_sb[:], bf2_d[:])
            nc.sync.dma_start(bg2_sb[:], bg2_d[:])
            nc.sync.dma_start(wh8_sb[:], wh8_d[:])
            st0["x4"] = sb_x.tile([128, RB * C], BF16, tag="x4", name="x4_t")
            st0["out4"] = sb_out.tile([128, RB * C], BF16, tag="out4",
                                      name="out4_t")
            sts[supers[0][0]] = st0
            for rb in supers[0][1:]:
                sts[rb] = start_dma(rb)
            nc.sync.dma_start(st0["x4"][:], x4_d[supers[0][0]])
            nc.sync.dma_start(onesg_sb[:], onesg_d[:])
            if len(supers) > 1:
                for rb in supers[1]:
                    sts[rb] = start_dma(rb)
            # Dummy matmuls bridging the runtime's ~9us DMA-startup delay:
            # trips the PE HAM activity window so real work starts at
            # 2.4GHz instead of the cold 1.2GHz default, and ends right
            # around when xt8(rb0)+wf+wg have landed.
            wu = cpool.tile([128, 128], BF16)
            nc.vector.memset(wu[:], 0.0)
            wu_ps = ps_s.tile([128, 258], F32, tag="s", name="wu_ps")
            for _ in range(60):
                nc.tensor.matmul(
                    wu_ps[0:8, 0:128], lhsT=wu[:, 0:8], rhs=wu[:, :],
                    start=True, stop=True,
                )
            banks = new_fg_banks()
            for rb in supers[0]:
                fg_mms(banks, sts[rb], last=(rb == supers[0][-1]))
            fg16 = {0: fg_drain(banks, len(supers[0]))}

            # global pair list: (super index, rb, pair)
            pair_seq = [
                (si, rb, p)
                for si, su in enumerate(supers)
                for rb in su
                for p in range(npair)
            ]
            pairs_per_super = len(supers[0]) * npair

            cur_st = sts[supers[0][0]]
            h2_next = emit_h(cur_st, 0)
            prev = None
            nbanks = banks
            for i, (si, rb, p) in enumerate(pair_seq):
                st = sts[rb]
                par = st["par"]
                ft, gt = fg16[si]
                h2_this = h2_next
                lo = 64 * par
                q = i % pairs_per_super
                nsi = si + 1
                nsu = supers[nsi] if nsi < len(supers) else None
                last_q = q == pairs_per_super - 1

                s_ps = ps_s.tile([128, 258], F32, tag="s", name="s_ps")
                for rr in range(2):
                    r = 2 * p + rr
                    nc.tensor.matmul(
                        s_ps[:, rr * 128 : (rr + 1) * 128],
                        lhsT=ft[lo:lo + 64, r], rhs=gt[lo:lo + 64, r],
                        start=True, stop=True,
                    )
                # next super's odd-rb f/g matmuls right behind the s-matmuls
                # so their drain (below, early in the ACT/DVE queues) lands
                # a full pair before the next super's first s-matmul
                if nsu and q == 2 and len(nsu) > 1:
                    fg_mms(nbanks, sts[nsu[1]], last=True)
                at2 = sb_at.tile([128, 256], BF16, tag="at2", name="at2_t")
                nc.scalar.activation(
                    at2[:], s_ps[:, 0:256], AF.Exp, scale=1.0 / (WS * WS)
                )
                if nsu and q == min(2, pairs_per_super - 1):
                    fg16[nsi] = fg_drain(nbanks, len(nsu))

                # stage upcoming supers: DMAs two supers ahead, f/g matmuls
                # one super ahead
                if nsu:
                    if q == 0 and si + 2 < len(supers):
                        sts[supers[si + 2][0]] = start_dma(supers[si + 2][0])
                    elif q == min(1, pairs_per_super - 1):
                        if si + 2 < len(supers) and len(supers[si + 2]) > 1:
                            sts[supers[si + 2][1]] = start_dma(supers[si + 2][1])
                        nbanks = new_fg_banks()
                        fg_mms(nbanks, sts[nsu[0]], last=(len(nsu) == 1))

                # prefetch next pair's h
                if i + 1 < len(pair_seq):
                    nsi2, nrb2, np2 = pair_seq[i + 1]
                    h2_next = emit_h(sts[nrb2], np2)
                else:
                    h2_next = None

                if prev is not None:
                    stage_b1(prev)
                    stage_b2(prev)
                prev = {"st": st, "p": p, "s_ps": s_ps, "at2": at2, "h2": h2_this}
            stage_b1(prev)
            stage_b2(prev)
    nc.compile()
    return nc


def make_in_map(x_b: np.ndarray, wf, bf, wg, bg, wh, bh, gamma) -> dict:
    """Host-side input staging for one core (layout/dtype + constant folds)."""
    x_b = np.asarray(x_b, np.float32)
    rows = x_b.shape[0]
    RB = row_batch(rows)
    nrb = rows // RB
    gamma_f = float(np.float32(np.asarray(gamma)))
    sgn = 1.0 if gamma_f >= 0 else -1.0
    ag = max(abs(gamma_f), 1e-30)

    xt8 = np.ascontiguousarray(
        x_b.astype(E4DT)
        .reshape(nrb, RB, W, 4, 128)
        .transpose(0, 4, 3, 1, 2)
        .reshape(nrb, 128, RB * C)
    )
    x_adj = x_b + gamma_f * np.asarray(bh, np.float32)
    x4 = np.ascontiguousarray(
        x_adj.astype(BFDT)
        .reshape(nrb, RB, W, C)
        .transpose(0, 2, 1, 3)
        .reshape(nrb, 128, RB * C)
    )

    def w_dr(w_mat, scale, pad=False):
        w_mat = np.asarray(w_mat, np.float32) * scale
        if pad:
            wp = np.zeros((w_mat.shape[0], 192), np.float32)
            wp[:, 64:128] = w_mat
            w_mat = wp
        m = w_mat.shape[1]
        return np.ascontiguousarray(
            w_mat.astype(E4DT).reshape(4, 128, m).transpose(1, 0, 2).reshape(128, 4 * m)
        )

    bf2 = np.concatenate([np.asarray(bf, np.float32)] * 2).reshape(128, 1) * WS
    bg2 = np.concatenate([np.asarray(bg, np.float32)] * 2).reshape(128, 1) * WS
    return {
        "xt8": xt8,
        "x4": x4,
        "wh8": w_dr(wh, WS * sgn),
        "wf8": w_dr(wf, WS, pad=True),
        "wg8": w_dr(wg, WS, pad=True),
        "bf2": bf2,
        "bg2": bg2,
        "onesg": np.full((W, 1), WS / ag, np.float32).astype(BFDT),
    }


def unbatch_out(arr: np.ndarray, rows: int) -> np.ndarray:
    """[nrb, 128, RB*C] device layout -> [rows, W, C] f32."""
    RB = row_batch(rows)
    nrb = rows // RB
    return (
        np.asarray(arr)
        .astype(np.float32)
        .reshape(nrb, 128, RB, C)
        .transpose(0, 2, 1, 3)
        .reshape(rows, W, C)
    )


_NC_CACHE: dict = {}


def run(inputs: dict, trace: bool = False, **run_kwargs):
    """Build (cached), run on 8 cores, return (out, BassKernelResults)."""
    from concourse.bass_utils import run_bass_kernel_spmd

    if "nc" not in _NC_CACHE:
        _NC_CACHE["nc"] = build_nc()
    nc = _NC_CACHE["nc"]
    x = np.asarray(inputs["x"], np.float32)
    in_maps = [
        make_in_map(
            x[b],
            inputs["wf"],
            inputs["bf"],
            inputs["wg"],
            inputs["bg"],
            inputs["wh"],
            inputs["bh"],
            inputs["gamma"],
        )
        for b in range(N_CORES)
    ]
    res = run_bass_kernel_spmd(
        nc, in_maps, list(range(N_CORES)), trace=trace, **run_kwargs
    )
    out = np.stack(
        [unbatch_out(res.results[b]["out"], H) for b in range(N_CORES)], axis=0
    )
    return out, res


def kernel(**inputs) -> np.ndarray:
    out, _ = run(inputs, trace=False)
    return out
